# revision 9
# baseline (speedup 1.0000x reference)
"""EnhancedGIN on 8 Trainium2 NeuronCores (Bass/Tile).

Strategy (SPMD, one program, per-core data):
 - Nodes partitioned contiguously: core c owns rows [c*6272, (c+1)*6272)
   (padded to 50176; pads zero).  Edges partitioned by dst owner.
 - Per layer: messages h[src] gathered from a replicated row-major bf16
   node table in HBM via dma_gather (int16 idx => table split in 2 halves);
   scatter-add by dst via one-hot (iota==dst_local) bf16 matmuls
   accumulated in PSUM per 128-node dst block, transposed layout
   aggT[feat, node].
 - MLP runs on the transposed layout: y1T = W1'^T x_inT without
   materializing x_in (distributed matmuls: W1s = (1+eps)W1, W1sx, tx*deg
   outer-product term folds input-BN of layer0 messages).
 - BatchNorm stats: local free-axis reductions + 1KB AllReduce; biases
   fold into the BN affine analytically.  Pad-node contamination removed
   via the uniform-pad-column correction (n_pad * y[:, last]).
 - After each layer: h written bf16 row-major (PE transpose) -> AllGather
   -> next layer's gather table.
 - JK attention per-node (scores via PE, softmax on [node,4] tiles),
   combine row-major; pooling: sums via one-hot matmul into a per-core
   128-graph window + scatter + AllReduce(add); max via per-graph padded
   index gather from local x_jk + chunked DVE max + scatter +
   AllReduce(max); head computed replicated on all cores.
"""

import sys

sys.path.insert(0, "/opt/trn_rl_repo")

import numpy as np
import ml_dtypes

bf16 = ml_dtypes.bfloat16

# problem constants (hardcoded per spec)
N = 50000
E = 600000
G = 512
H = 128
LAT = 64
L = 4
BN_EPS = 1e-5
NCORES = 8
P = 128
NPC = 6272  # nodes per core (49 blocks of 128)
NB = NPC // P  # 49
NPAD = NCORES * NPC  # 50176
HALF = NPAD // 2  # 25088
NEG = -1.0e30


def _wrap_idx(flat):
    """int16 flat index list (len%16==0) -> [128, len/16] wrapped layout."""
    assert len(flat) % 16 == 0
    w = flat.reshape(-1, 16).T.astype(np.int16)  # [16, len/16]
    return np.tile(w, (8, 1)).copy()  # [128, len/16]


def preprocess(x, edge_index, batch):
    """Host-side sharding. Returns (consts, percore) -- consts are
    compile-time (identical across cores), percore is a list of dicts."""
    x = np.asarray(x, np.float32)
    src = np.asarray(edge_index[0], np.int64)
    dst = np.asarray(edge_index[1], np.int64)
    batch = np.asarray(batch, np.int64)

    owner = dst // NPC
    blk = (dst % NPC) // P
    dloc = dst % P
    half = (src >= HALF).astype(np.int64)

    # group edges by (owner, blk, half)
    key = ((owner * NB + blk) * 2 + half).astype(np.int64)
    order = np.argsort(key, kind="stable")
    key_s = key[order]
    src_s = src[order]
    dloc_s = dloc[order]
    counts = np.bincount(key_s, minlength=NCORES * NB * 2).reshape(NCORES, NB, 2)
    TL = int(np.ceil(counts[:, :, 0].max() / P))
    TH = int(np.ceil(counts[:, :, 1].max() / P))
    T = TL + TH
    starts = np.concatenate([[0], np.cumsum(counts.reshape(-1))]).astype(np.int64)

    deg = np.bincount(dst, minlength=N).astype(np.float32)
    deg_pad = np.zeros(NPAD, np.float32)
    deg_pad[:N] = deg

    cnt = np.bincount(batch, minlength=G).astype(np.float32)

    # graph window bases
    gbase = []
    for c in range(NCORES):
        s = c * NPC
        gbase.append(int(batch[min(s, N - 1)]))
    for c in range(NCORES):
        e_ = min((c + 1) * NPC, N) - 1
        span = int(batch[e_]) - gbase[c]
        assert span < P, f"graph window span {span} >= 128 on core {c}"

    # per-graph padded node lists (local indices), W slots
    Wmax = int(cnt.max())
    W = ((Wmax + 7) // 8) * 8
    CH = W // 8

    percore = []
    for c in range(NCORES):
        lo_idx = np.zeros((NB, TL * P), np.int64)
        hi_idx = np.zeros((NB, TH * P), np.int64)
        dstloc = np.full((NB, T * P), -1.0, np.float32)
        for b in range(NB):
            k0 = (c * NB + b) * 2
            s0, e0 = starts[k0], starts[k0 + 1]
            s1, e1 = starts[k0 + 1], starts[k0 + 2]
            nlo, nhi = e0 - s0, e1 - s1
            lo_idx[b, :nlo] = src_s[s0:e0]
            hi_idx[b, :nhi] = src_s[s1:e1] - HALF
            dstloc[b, :nlo] = dloc_s[s0:e0]
            dstloc[b, TL * P : TL * P + nhi] = dloc_s[s1:e1]
        # wrapped int16 idx arrays, concatenated per block
        idxlo = np.concatenate([_wrap_idx(lo_idx[b]) for b in range(NB)], axis=1)
        idxhi = np.concatenate([_wrap_idx(hi_idx[b]) for b in range(NB)], axis=1)
        # dst as [128, NB*T] f32 (per-tile column)
        dstf = dstloc.reshape(NB * T, P).T.copy().astype(np.float32)

        n0 = c * NPC
        nreal = max(0, min(NPC, N - n0))
        xs = np.zeros((NPC, H), np.float32)
        xs[:nreal] = x[n0 : n0 + nreal]

        batch_loc = np.full(NPC, -1.0, np.float32)
        bl = batch[n0 : n0 + nreal] - gbase[c]
        batch_loc[:nreal] = bl.astype(np.float32)
        assert nreal == 0 or (bl.min() >= 0 and bl.max() < P)
        batchloc = batch_loc.reshape(NB, P).T.copy().astype(np.float32)  # [128, NB]

        gscat = (gbase[c] + np.arange(P)).astype(np.int32).reshape(P, 1)

        # per-local-graph padded node lists -> wrapped gather idx
        # local graph g = gbase[c]+g ; nodes are local row ids; pads -> NPC (spare)
        nl = np.full((P, W), NPC, np.int64)
        for g in range(P):
            gid = gbase[c] + g
            if gid >= G:
                continue
            rows = np.nonzero(batch[n0 : n0 + nreal] == gid)[0]
            nl[g, : len(rows)] = rows
        # chunk ch covers slots [ch*16,(ch+1)*16); flat[i] = nl[i%128, ch*16+i//128]
        mw = []
        for ch in range(CH):
            sl = nl[:, ch * 8 : (ch + 1) * 8]  # [128 g, 8]
            flat = sl.T.reshape(-1)  # i = s*128+g, 1024 idxs
            mw.append(_wrap_idx(flat))
        maxidx = np.concatenate(mw, axis=1)  # [128, CH*64]

        npad_c = float(NPC - nreal)
        percore.append(
            dict(
                idxlo=idxlo.astype(np.int16),
                idxhi=idxhi.astype(np.int16),
                dstf=dstf,
                xT=xs.T.copy(),  # [128, NPC] f32
                deg_row=deg_pad[n0 : n0 + NPC].reshape(1, NPC).copy(),
                batchloc=batchloc,
                gscat=gscat,
                maxidx=maxidx.astype(np.int16),
                n_pad=np.full((P, 1), npad_c, np.float32),
            )
        )

    x_raw = np.zeros((NPAD, H), np.float32)
    x_raw[:N] = x
    consts = dict(TL=TL, TH=TH, T=T, W=W, CH=CH, gbase=gbase, cnt=cnt)
    return consts, percore, x_raw.astype(bf16)


# ---------------------------------------------------------------------------
# numpy mirror of the device algorithm (bf16 rounding where the device uses
# bf16).  Used for validation only.
# ---------------------------------------------------------------------------


def _np_gelu(v):
    from scipy.special import erf

    return 0.5 * v * (1.0 + erf(v / np.sqrt(2.0)))


def numpy_sim(consts, percore, x_raw, weights):
    TL, TH, T = consts["TL"], consts["TH"], consts["T"]
    W, CH = consts["W"], consts["CH"]
    gbase = consts["gbase"]
    cnt = consts["cnt"]

    eps = weights["eps"]
    fc1_w, fc2_w = weights["fc1_w"], weights["fc2_w"]
    bn1_g, bn1_b = weights["bn1_g"], weights["bn1_b"]
    bn_g, bn_b = weights["bn_g"], weights["bn_b"]
    att = weights["att_weight"]

    # input BN stats from xT shards
    s_loc = np.zeros((NCORES, H), np.float64)
    q_loc = np.zeros((NCORES, H), np.float64)
    for c in range(NCORES):
        xT = percore[c]["xT"].astype(np.float64)
        s_loc[c] = xT.sum(1)
        q_loc[c] = (xT * xT).sum(1)
    sg = s_loc.sum(0).astype(np.float32)
    qg = q_loc.sum(0).astype(np.float32)
    m = sg / N
    var = qg / N - m * m
    sx = weights["input_bn_g"] / np.sqrt(var + BN_EPS)
    tx = weights["input_bn_b"] - m * sx

    hT = [
        (sx[:, None] * percore[c]["xT"] + tx[:, None]).astype(np.float32)
        for c in range(NCORES)
    ]
    tbl = x_raw  # bf16 [NPAD, H]
    hjk = [[None] * L for _ in range(NCORES)]

    for l in range(L):
        zs = []
        for c in range(NCORES):
            pc = percore[c]
            # aggregation
            aggT = np.zeros((H, NPC), np.float32)
            for b in range(NB):
                acc = np.zeros((H, P), np.float32)
                for t in range(T):
                    if t < TL:
                        w = pc["idxlo"][:16, (b * TL + t) * 8 : (b * TL + t + 1) * 8]
                        idx = w.T.reshape(-1).astype(np.int64)
                        msg = tbl[idx]  # bf16 [128, H]
                    else:
                        th = t - TL
                        w = pc["idxhi"][:16, (b * TH + th) * 8 : (b * TH + th + 1) * 8]
                        idx = w.T.reshape(-1).astype(np.int64) + HALF
                        msg = tbl[idx]
                    d = pc["dstf"][:, b * T + t]  # [128]
                    S = (np.arange(P)[None, :] == d[:, None]).astype(bf16)
                    acc += msg.astype(np.float32).T @ S.astype(np.float32)
                aggT[:, b * P : (b + 1) * P] = acc
            if l == 0:
                W1s = (1.0 + eps[0]) * fc1_w[0]
                W1sx = sx[:, None] * fc1_w[0]
                txW = fc1_w[0].T @ tx  # [H]
                z = (
                    W1s.T @ hT[c]
                    + W1sx.T @ aggT
                    + txW[:, None] * pc["deg_row"][0][None, :]
                )
            else:
                z = (1.0 + eps[l]) * (fc1_w[l].T @ hT[c]) + fc1_w[l].T @ aggT
            zs.append(z)
        # BN1 (bias folds away)
        s_l = np.zeros(H, np.float64)
        q_l = np.zeros(H, np.float64)
        for c in range(NCORES):
            zp = zs[c][:, -1]
            npad = percore[c]["n_pad"][0, 0]
            s_l += zs[c].sum(1) - npad * zp
            q_l += (zs[c] ** 2).sum(1) - npad * zp * zp
        mz = (s_l / N).astype(np.float32)
        vz = (q_l / N).astype(np.float32) - mz * mz
        s1 = bn1_g[l] / np.sqrt(vz + BN_EPS)
        cb1 = bn1_b[l] - mz * s1
        a1 = [
            _np_gelu(s1[:, None] * zs[c] + cb1[:, None]).astype(bf16)
            for c in range(NCORES)
        ]
        # fc2 (bf16 matmul)
        z2 = [
            (fc2_w[l].astype(bf16).astype(np.float32).T)
            @ a1[c].astype(np.float32)
            for c in range(NCORES)
        ]
        s_l = np.zeros(H, np.float64)
        q_l = np.zeros(H, np.float64)
        for c in range(NCORES):
            zp = z2[c][:, -1]
            npad = percore[c]["n_pad"][0, 0]
            s_l += z2[c].sum(1) - npad * zp
            q_l += (z2[c] ** 2).sum(1) - npad * zp * zp
        mz = (s_l / N).astype(np.float32)
        vz = (q_l / N).astype(np.float32) - mz * mz
        s2 = bn_g[l] / np.sqrt(vz + BN_EPS)
        cb2 = bn_b[l] - mz * s2
        newh = [
            _np_gelu(s2[:, None] * z2[c] + cb2[:, None]).astype(np.float32)
            for c in range(NCORES)
        ]
        for c in range(NCORES):
            hjk[c][l] = newh[c].astype(bf16)
            hT[c] = newh[c]
        if l < L - 1:
            tbl = np.concatenate([newh[c].T for c in range(NCORES)], 0).astype(bf16)

    # JK + pooling
    pool_sum = np.zeros((640, H), np.float32)
    pool_max = np.full((640, H), NEG, np.float32)
    outs = []
    for c in range(NCORES):
        pc = percore[c]
        sc = np.zeros((NPC, L), np.float32)
        for l in range(L):
            sc[:, l] = (
                hjk[c][l].astype(np.float32).T @ (att[l] / H)
            )
        a = np.exp(sc - sc.max(1, keepdims=True))
        a /= a.sum(1, keepdims=True)
        xjk = np.zeros((NPC, H), np.float32)
        for l in range(L):
            xjk += a[:, l : l + 1] * hjk[c][l].astype(np.float32).T
        # sums via one-hot in window
        psum = np.zeros((P, H), np.float32)
        for b in range(NB):
            d = pc["batchloc"][:, b]
            Gm = (np.arange(P)[None, :] == d[:, None]).astype(np.float32)
            psum += Gm.T @ xjk[b * P : (b + 1) * P]
        sl = pool_sum[gbase[c] : gbase[c] + P]
        sl += psum
        # max via padded gather
        xjk_d = np.concatenate([xjk, np.full((P, H), NEG, np.float32)], 0)
        gm = np.full((P, H), NEG, np.float32)
        for ch in range(CH):
            w = pc["maxidx"][:16, ch * 64 : (ch + 1) * 64]
            idx = w.T.reshape(-1).astype(np.int64)
            gath = xjk_d[idx].reshape(8, P, H)  # slot-major
            gm = np.maximum(gm, gath.max(0))
        ml = pool_max[gbase[c] : gbase[c] + P]
        np.maximum(ml, gm, out=ml)

    # head (identical on all cores)
    sums = pool_sum[:G]
    mx = pool_max[:G]
    cntc = np.maximum(cnt, 1.0)
    mean = sums / cntc[:, None]
    mask = (cnt > 0).astype(np.float32)
    mxf = mx * mask[:, None]
    pwx = weights["pool_weight"]
    pw = np.exp(pwx - pwx.max())
    pw /= pw.sum()
    pooled = sums * pw[0] + mean * pw[1] + mxf * pw[2]
    out1 = pooled @ weights["fcA_w"] + weights["fcA_b"]
    mu = out1.mean(1, keepdims=True)
    vv = out1.var(1)
    z = (out1 - mu) / np.sqrt(vv[:, None] + 1e-5)
    gg = _np_gelu(z * weights["ln_g"] + weights["ln_b"]) + pooled
    return gg @ weights["fcB_w"] + weights["fcB_b"]


# ---------------------------------------------------------------------------
# Bass/Tile device program
# ---------------------------------------------------------------------------

CHUNKS = [(i * 512, min((i + 1) * 512, NPC)) for i in range((NPC + 511) // 512)]


def build_bass(consts, taps=False):
    import concourse.bass as bass
    import concourse.bacc as bacc
    import concourse.mybir as mybir
    import concourse.tile as tile

    dt = mybir.dt
    Alu = mybir.AluOpType
    Act = mybir.ActivationFunctionType
    TL, TH, T, W, CH = (
        consts["TL"],
        consts["TH"],
        consts["T"],
        consts["W"],
        consts["CH"],
    )
    rg = [list(range(NCORES))]

    nc = bacc.Bacc("TRN2", target_bir_lowering=False, num_devices=NCORES)

    # ---- external inputs -------------------------------------------------
    def ein(name, shape, dtype):
        return nc.dram_tensor(name, shape, dtype, kind="ExternalInput")

    idxlo_d = ein("idxlo", [P, NB * TL * 8], dt.int16)
    idxhi_d = ein("idxhi", [P, NB * TH * 8], dt.int16)
    dstf_d = ein("dstf", [P, NB * T], dt.float32)
    xT_d = ein("xT", [P, NPC], dt.float32)
    deg_d = ein("deg_row", [1, NPC], dt.float32)
    batchloc_d = ein("batchloc", [P, NB], dt.float32)
    gscat_d = ein("gscat", [P, 1], dt.int32)
    maxidx_d = ein("maxidx", [P, CH * 64], dt.int16)
    npad_d = ein("n_pad", [P, 1], dt.float32)
    xraw_d = ein("x_raw", [NPAD, H], dt.bfloat16)
    w1_d = ein("w1", [P, L * H], dt.float32)
    w2_d = ein("w2", [P, L * H], dt.float32)
    vecs_d = ein("vecs", [P, 21], dt.float32)
    attT_d = ein("attT", [P, L], dt.float32)
    eps_d = ein("eps_row", [1, L], dt.float32)
    poolw_d = ein("poolw", [1, 3], dt.float32)
    fcA_d = ein("fcA", [H, H], dt.float32)
    fcB_d = ein("fcB", [H, LAT], dt.float32)
    fcBb_d = ein("fcBb", [LAT, 1], dt.float32)
    cnt_d = ein("cnt_col", [P, 4], dt.float32)
    ones_d = ein("ones_row", [1, P], dt.float32)
    iota_f_d = ein("iota_f", [P, P], dt.float32)
    iota_b_d = ein("iota_b", [P, P], dt.bfloat16)
    ident_f_d = ein("ident_f", [P, P], dt.float32)
    ident_b_d = ein("ident_b", [P, P], dt.bfloat16)

    out_d = nc.dram_tensor("out", [G, LAT], dt.float32, kind="ExternalOutput")
    tap_d = {}
    if taps:
        tap_d["z1"] = nc.dram_tensor("tap_z1", [P, NPC], dt.float32, kind="ExternalOutput")
        tap_d["h1"] = nc.dram_tensor("tap_h1", [P, NPC], dt.float32, kind="ExternalOutput")
        tap_d["alpha"] = nc.dram_tensor("tap_alpha", [NPC, L], dt.float32, kind="ExternalOutput")
        tap_d["pool"] = nc.dram_tensor("tap_pool", [P, H], dt.float32, kind="ExternalOutput")
        tap_d["gmax"] = nc.dram_tensor("tap_gmax", [P, H], dt.float32, kind="ExternalOutput")

    # ---- internal DRAM ---------------------------------------------------
    hfull = [
        nc.dram_tensor(f"hfull{i}", [NPAD, H], dt.bfloat16, addr_space="Shared")
        for i in range(2)
    ]
    ag_in = nc.dram_tensor("ag_in", [NPC, H], dt.bfloat16)
    stat_in = [nc.dram_tensor(f"stin{k}", [P, 2], dt.float32) for k in range(9)]
    stat_out = [
        nc.dram_tensor(f"stout{k}", [P, 2], dt.float32, addr_space="Shared")
        for k in range(9)
    ]
    pool_in = nc.dram_tensor("pool_in", [640, H], dt.float32)
    pool_out = nc.dram_tensor("pool_out", [640, H], dt.float32, addr_space="Shared")
    max_in = nc.dram_tensor("max_in", [640, H], dt.float32)
    max_out = nc.dram_tensor("max_out", [640, H], dt.float32, addr_space="Shared")
    xjk_dram = nc.dram_tensor("xjk_dram", [NPC + P, H], dt.float32)
    hjk_dram = [
        nc.dram_tensor(f"hjkd{l}", [P, NPC], dt.bfloat16) for l in range(L)
    ]

    with tile.TileContext(nc) as tc:
        _build_body(
            nc, tc, bass, mybir, consts, locals()
        )
    nc.compile()
    return nc


def _build_body(nc, tc, bass, mybir, consts, env):
    dt = mybir.dt
    Alu = mybir.AluOpType
    Act = mybir.ActivationFunctionType
    TL, TH, T, W, CH = (
        consts["TL"],
        consts["TH"],
        consts["T"],
        consts["W"],
        consts["CH"],
    )
    rg = [list(range(NCORES))]
    g = lambda k: env[k]
    taps = g("taps")
    tap_d = g("tap_d")
    PH = consts.get("PH", 99)
    NLAYERS = min(L, PH) if PH < 5 else L

    with (
        tc.tile_pool(name="const", bufs=1) as C,
        tc.tile_pool(name="work", bufs=3) as WK,
        tc.tile_pool(name="mgp", bufs=2) as MG,
        tc.tile_pool(name="vwork", bufs=10) as VW,
    ):
        # ---------- persistent SBUF loads ---------------------------------
        def load(dram, shape, dtype, tag):
            t = C.tile(shape, dtype, tag=tag, name=tag)
            nc.sync.dma_start(out=t[:], in_=dram[:])
            return t

        idxlo = load(g("idxlo_d"), [P, NB * TL * 8], dt.int16, "idxlo")
        idxhi = load(g("idxhi_d"), [P, NB * TH * 8], dt.int16, "idxhi")
        dstf = load(g("dstf_d"), [P, NB * T], dt.float32, "dstf")
        deg = load(g("deg_d"), [1, NPC], dt.float32, "deg")
        batchloc = load(g("batchloc_d"), [P, NB], dt.float32, "batchloc")
        gscat = load(g("gscat_d"), [P, 1], dt.int32, "gscat")
        maxidx = load(g("maxidx_d"), [P, CH * 64], dt.int16, "maxidx")
        npad = load(g("npad_d"), [P, 1], dt.float32, "npad")
        w1 = load(g("w1_d"), [P, L * H], dt.float32, "w1")
        w2 = load(g("w2_d"), [P, L * H], dt.float32, "w2")
        vecs = load(g("vecs_d"), [P, 21], dt.float32, "vecs")
        attT = load(g("attT_d"), [P, L], dt.float32, "attT")
        eps_r = load(g("eps_d"), [1, L], dt.float32, "eps")
        poolw = load(g("poolw_d"), [1, 3], dt.float32, "poolw")
        fcA = load(g("fcA_d"), [H, H], dt.float32, "fcA")
        fcB = load(g("fcB_d"), [H, LAT], dt.float32, "fcB")
        fcBb = load(g("fcBb_d"), [LAT, 1], dt.float32, "fcBb")
        cntc = load(g("cnt_d"), [P, 4], dt.float32, "cnt")
        ones_r = load(g("ones_d"), [1, P], dt.float32, "ones")
        iota_f = load(g("iota_f_d"), [P, P], dt.float32, "iotaf")
        iota_b = load(g("iota_b_d"), [P, P], dt.bfloat16, "iotab")
        ident_f = load(g("ident_f_d"), [P, P], dt.float32, "identf")
        ident_b = load(g("ident_b_d"), [P, P], dt.bfloat16, "identb")

        hT = load(g("xT_d"), [P, NPC], dt.float32, "hT")  # becomes x0T in-place
        y_sb = C.tile([P, NPC], dt.float32, tag="ysb")
        a1T = C.tile([P, NPC], dt.bfloat16, tag="a1T")
        hjk_dram = g("hjk_dram")

        epsc = C.tile([P, 1], dt.float32, tag="epsc", name="epsc")
        nc.vector.memset(epsc[:], BN_EPS)

        # vector scratch (all [128,1] or [128,k])
        def vt(w=1, dtype=dt.float32):
            return VW.tile([P, w], dtype, tag=f"v{w}", name=f"v{w}")

        # column helper
        col = lambda t, j: t[:, j : j + 1]

        V = nc.vector
        S = nc.scalar
        PE = nc.tensor

        # ---------- collective helper -------------------------------------
        def allreduce(k, sb_in, cols=2, op=Alu.add):
            nc.sync.dma_start(out=g("stat_in")[k][:, :cols], in_=sb_in[:, :cols])
            nc.gpsimd.collective_compute(
                "AllReduce", op, replica_groups=rg,
                ins=[g("stat_in")[k][:]], outs=[g("stat_out")[k][:]],
            )
            o = VW.tile([P, 2], dt.float32, tag="arout")
            nc.sync.dma_start(out=o[:, :cols], in_=g("stat_out")[k][:, :cols])
            return o

        # ---------- BN stat finalize: returns (scale, cbias) --------------
        def bn_finalize(k, spart, qpart, nparts, gcol, bcol, zp=None):
            sl = vt()
            ql = vt()
            V.reduce_sum(out=sl[:], in_=spart[:, :nparts], axis=mybir.AxisListType.X)
            V.reduce_sum(out=ql[:], in_=qpart[:, :nparts], axis=mybir.AxisListType.X)
            if zp is not None:
                t0 = vt()
                V.tensor_scalar(out=t0[:], in0=zp, scalar1=npad[:, :1], scalar2=None, op0=Alu.mult)
                V.tensor_tensor(out=sl[:], in0=sl[:], in1=t0[:], op=Alu.subtract)
                zp2 = vt()
                V.tensor_tensor(out=zp2[:], in0=zp, in1=zp, op=Alu.mult)
                V.tensor_scalar(out=zp2[:], in0=zp2[:], scalar1=npad[:, :1], scalar2=None, op0=Alu.mult)
                V.tensor_tensor(out=ql[:], in0=ql[:], in1=zp2[:], op=Alu.subtract)
            stat = VW.tile([P, 2], dt.float32, tag="statin")
            V.tensor_copy(out=col(stat, 0), in_=sl[:])
            V.tensor_copy(out=col(stat, 1), in_=ql[:])
            arr = allreduce(k, stat)
            mz = vt()
            V.tensor_scalar(out=mz[:], in0=col(arr, 0), scalar1=1.0 / N, scalar2=None, op0=Alu.mult)
            ex2 = vt()
            V.tensor_scalar(out=ex2[:], in0=col(arr, 1), scalar1=1.0 / N, scalar2=None, op0=Alu.mult)
            var = vt()
            V.tensor_tensor(out=var[:], in0=mz[:], in1=mz[:], op=Alu.mult)
            V.tensor_tensor(out=var[:], in0=ex2[:], in1=var[:], op=Alu.subtract)
            std = vt()
            S.activation(out=std[:], in_=var[:], func=Act.Sqrt, bias=epsc[:, :1])
            rstd = vt()
            V.reciprocal(out=rstd[:], in_=std[:])
            sc = VW.tile([P, 1], dt.float32, tag="bnsc")
            V.tensor_tensor(out=sc[:], in0=gcol, in1=rstd[:], op=Alu.mult)
            cb = VW.tile([P, 1], dt.float32, tag="bncb")
            V.tensor_tensor(out=cb[:], in0=mz[:], in1=sc[:], op=Alu.mult)
            V.tensor_tensor(out=cb[:], in0=bcol, in1=cb[:], op=Alu.subtract)
            return sc, cb

        with (
            tc.tile_pool(name="psA", bufs=2, space="PSUM") as PSA,
            tc.tile_pool(name="psZ", bufs=2, space="PSUM") as PSZ,
            tc.tile_pool(name="psB", bufs=2, space="PSUM") as PSB,
            tc.tile_pool(name="psP", bufs=1, space="PSUM") as PSP,
        ):
            # ---------- input BN ------------------------------------------
            spart = C.tile([P, 16], dt.float32, tag="spart")
            qpart = C.tile([P, 16], dt.float32, tag="qpart")
            for i, (a, b) in enumerate(CHUNKS):
                V.reduce_sum(out=col(spart, i), in_=hT[:, a:b], axis=mybir.AxisListType.X)
                sq_scr = WK.tile([P, 512], dt.float32, tag="sqscr")
                S.activation(out=sq_scr[:, : b - a], in_=hT[:, a:b], func=Act.Square,
                             accum_out=col(qpart, i))
            sx, tx = bn_finalize(0, spart, qpart, len(CHUNKS), col(vecs, 0), col(vecs, 1))
            for a, b in CHUNKS:
                V.tensor_scalar(out=hT[:, a:b], in0=hT[:, a:b], scalar1=sx[:, :1],
                                scalar2=tx[:, :1], op0=Alu.mult, op1=Alu.add)

            # epsp1 broadcast [128, L]
            psb = PSB.tile([P, L], dt.float32, space="PSUM", tag="ps_b")
            PE.matmul(out=psb[:], lhsT=ones_r[:], rhs=eps_r[:], start=True, stop=True)
            epsp1 = C.tile([P, L], dt.float32, tag="epsp1")
            V.tensor_scalar(out=epsp1[:], in0=psb[:], scalar1=1.0, scalar2=None, op0=Alu.add)

            # layer-0 input-BN folding terms
            w1x0 = C.tile([H, H], dt.float32, tag="w1x0")
            V.tensor_scalar(out=w1x0[:], in0=w1[:, 0:H], scalar1=sx[:, :1], scalar2=None, op0=Alu.mult)
            pstxw = PSB.tile([P, 1], dt.float32, space="PSUM", tag="ps_b")
            PE.matmul(out=pstxw[:], lhsT=w1[:, 0:H], rhs=tx[:, :1], start=True, stop=True)
            txw_c = C.tile([P, 1], dt.float32, tag="txwc")
            V.tensor_copy(out=txw_c[:], in_=pstxw[:])
            pstxr = PSB.tile([1, P], dt.float32, space="PSUM", tag="ps_b")
            PE.transpose(out=pstxr[:], in_=txw_c[:], identity=ident_f[:])
            txw_r = C.tile([1, P], dt.float32, tag="txwr")
            V.tensor_copy(out=txw_r[:], in_=pstxr[:])

            def _dummy_out():
                zt_ = WK.tile([P, LAT], dt.float32, tag="dummy")
                nc.vector.memset(zt_[:], 0.0)
                for tt_ in range(4):
                    nc.sync.dma_start(out=g("out_d")[tt_ * P : (tt_ + 1) * P, :], in_=zt_[:])

            # ---------- layers --------------------------------------------
            for l in range(NLAYERS):
                w1s = WK.tile([H, H], dt.float32, tag="w1s")
                V.tensor_scalar(out=w1s[:], in0=w1[:, l * H : (l + 1) * H],
                                scalar1=col(epsp1, l), scalar2=None, op0=Alu.mult)
                w2b = WK.tile([H, H], dt.bfloat16, tag="w2b")
                V.tensor_copy(out=w2b[:], in_=w2[:, l * H : (l + 1) * H])
                if l == 0:
                    tbl_lo = g("xraw_d")[0:HALF, :]
                    tbl_hi = g("xraw_d")[HALF:NPAD, :]
                else:
                    hf = g("hfull")[(l - 1) % 2]
                    tbl_lo = hf[0:HALF, :]
                    tbl_hi = hf[HALF:NPAD, :]

                ngrp = len(CHUNKS)
                for gi, (ga, gb) in enumerate(CHUNKS):
                    nblk = (gb - ga) // P
                    agg_sb = WK.tile([P, 512], dt.float32, tag="aggsb")
                    for bi in range(nblk):
                        b = ga // P + bi
                        psA = PSA.tile([P, P], dt.float32, space="PSUM", tag="ps_a")
                        mlo = WK.tile([P, TL, P], dt.bfloat16, tag="mlo")
                        nc.gpsimd.dma_gather(
                            out_ap=mlo[:], in_ap=tbl_lo,
                            idxs_ap=idxlo[:, b * TL * 8 : (b + 1) * TL * 8],
                            num_idxs=TL * P, num_idxs_reg=TL * P, elem_size=P,
                        )
                        mhi = WK.tile([P, TH, P], dt.bfloat16, tag="mhi")
                        nc.gpsimd.dma_gather(
                            out_ap=mhi[:], in_ap=tbl_hi,
                            idxs_ap=idxhi[:, b * TH * 8 : (b + 1) * TH * 8],
                            num_idxs=TH * P, num_idxs_reg=TH * P, elem_size=P,
                        )
                        for t in range(T):
                            st = WK.tile([P, P], dt.bfloat16, tag="onehot")
                            V.tensor_scalar(out=st[:], in0=iota_b[:],
                                            scalar1=col(dstf, b * T + t), scalar2=None,
                                            op0=Alu.is_equal)
                            mv = mlo[:, t, :] if t < TL else mhi[:, t - TL, :]
                            PE.matmul(out=psA[:], lhsT=mv, rhs=st[:],
                                      start=(t == 0), stop=(t == T - 1))
                        V.tensor_copy(out=agg_sb[:, bi * P : (bi + 1) * P], in_=psA[:])
                    w = gb - ga
                    psZ = PSZ.tile([P, 512], dt.float32, space="PSUM", tag="ps_z")
                    PE.matmul(out=psZ[:, :w], lhsT=w1s[:], rhs=hT[:, ga:gb],
                              start=True, stop=False)
                    lhs2 = w1x0 if l == 0 else w1[:, l * H : (l + 1) * H]
                    PE.matmul(out=psZ[:, :w], lhsT=(lhs2[:] if l == 0 else lhs2), rhs=agg_sb[:, :w],
                              start=False, stop=(l != 0))
                    if l == 0:
                        PE.matmul(out=psZ[:, :w], lhsT=txw_r[:], rhs=deg[:, ga:gb],
                                  start=False, stop=True)
                    V.reduce_sum(out=col(spart, gi), in_=psZ[:, :w], axis=mybir.AxisListType.X)
                    sq_scr = WK.tile([P, 512], dt.float32, tag="sqscr")
                    S.activation(out=sq_scr[:, :w], in_=psZ[:, :w], func=Act.Square,
                                 accum_out=col(qpart, gi))
                    V.tensor_copy(out=y_sb[:, ga:gb], in_=psZ[:, :w])

                if taps and l == 0:
                    nc.sync.dma_start(out=tap_d["z1"][:], in_=y_sb[:])
                s1, cb1 = bn_finalize(1 + 2 * l, spart, qpart, ngrp,
                                      col(vecs, 2 + l), col(vecs, 6 + l),
                                      zp=y_sb[:, NPC - 1 : NPC])
                for a, b in CHUNKS:
                    S.activation(out=a1T[:, a:b], in_=y_sb[:, a:b], func=Act.Gelu,
                                 bias=cb1[:, :1], scale=s1[:, :1])
                for gi, (ga, gb) in enumerate(CHUNKS):
                    w = gb - ga
                    psZ = PSZ.tile([P, 512], dt.float32, space="PSUM", tag="ps_z")
                    PE.matmul(out=psZ[:, :w], lhsT=w2b[:], rhs=a1T[:, ga:gb],
                              start=True, stop=True)
                    V.reduce_sum(out=col(spart, gi), in_=psZ[:, :w], axis=mybir.AxisListType.X)
                    sq_scr = WK.tile([P, 512], dt.float32, tag="sqscr")
                    S.activation(out=sq_scr[:, :w], in_=psZ[:, :w], func=Act.Square,
                                 accum_out=col(qpart, gi))
                    V.tensor_copy(out=y_sb[:, ga:gb], in_=psZ[:, :w])
                s2, cb2 = bn_finalize(2 + 2 * l, spart, qpart, ngrp,
                                      col(vecs, 10 + l), col(vecs, 14 + l),
                                      zp=y_sb[:, NPC - 1 : NPC])
                for a, b in CHUNKS:
                    S.activation(out=hT[:, a:b], in_=y_sb[:, a:b], func=Act.Gelu,
                                 bias=cb2[:, :1], scale=s2[:, :1])
                nc.gpsimd.dma_start(out=hjk_dram[l][:], in_=hT[:])
                if taps and l == 1:
                    nc.sync.dma_start(out=tap_d["h1"][:], in_=hT[:])
                if l < L - 1:
                    for b in range(NB):
                        pst = PSA.tile([P, P], dt.float32, space="PSUM", tag="ps_a")
                        PE.transpose(out=pst[:], in_=hT[:, b * P : (b + 1) * P],
                                     identity=ident_f[:])
                        hrow = WK.tile([P, P], dt.bfloat16, tag="hrow")
                        V.tensor_copy(out=hrow[:], in_=pst[:])
                        nc.sync.dma_start(out=g("ag_in")[b * P : (b + 1) * P, :], in_=hrow[:])
                    nc.gpsimd.collective_compute(
                        "AllGather", Alu.bypass, replica_groups=rg,
                        ins=[g("ag_in")[:]], outs=[g("hfull")[l % 2][:]],
                    )

            if PH < 5:
                _dummy_out()
                return
            # ---------- init DRAM buffers for pooling ---------------------
            zero_sb = WK.tile([P, H], dt.float32, tag="zfill")
            nc.vector.memset(zero_sb[:], 0.0)
            ninf_sb = WK.tile([P, H], dt.float32, tag="nfill")
            nc.vector.memset(ninf_sb[:], NEG)
            for tt in range(5):
                nc.sync.dma_start(out=g("pool_in")[tt * P : (tt + 1) * P, :], in_=zero_sb[:])
                nc.sync.dma_start(out=g("max_in")[tt * P : (tt + 1) * P, :], in_=ninf_sb[:])
            nc.sync.dma_start(out=g("xjk_dram")[NPC : NPC + P, :], in_=ninf_sb[:])

            # attention weights, scaled
            att_b = C.tile([P, L], dt.bfloat16, tag="attb")
            V.tensor_scalar(out=att_b[:], in0=attT[:], scalar1=1.0 / H, scalar2=None, op0=Alu.mult)

            # ---------- JK + sum-pooling ----------------------------------
            psP = PSP.tile([P, H], dt.float32, space="PSUM", tag="ps_p")
            for b in range(NB):
                blk = slice(b * P, (b + 1) * P)
                hjt = []
                for l in range(L):
                    t_ = WK.tile([P, P], dt.bfloat16, tag=f"hjt{l}")
                    nc.sync.dma_start(out=t_[:], in_=hjk_dram[l][:, blk])
                    hjt.append(t_)
                psS = PSB.tile([P, L], dt.float32, space="PSUM", tag="ps_b")
                for l in range(L):
                    PE.matmul(out=psS[:, l : l + 1], lhsT=hjt[l][:],
                              rhs=att_b[:, l : l + 1], start=True, stop=True,
                              skip_group_check=True)
                rmax = vt()
                V.reduce_max(out=rmax[:], in_=psS[:], axis=mybir.AxisListType.X)
                nmax = vt()
                V.tensor_scalar(out=nmax[:], in0=rmax[:], scalar1=-1.0, scalar2=None, op0=Alu.mult)
                esc = WK.tile([P, L], dt.float32, tag="esc")
                S.activation(out=esc[:], in_=psS[:], func=Act.Exp, bias=nmax[:, :1])
                rsum = vt()
                V.reduce_sum(out=rsum[:], in_=esc[:], axis=mybir.AxisListType.X)
                rcp = vt()
                V.reciprocal(out=rcp[:], in_=rsum[:])
                alpha = WK.tile([P, L], dt.float32, tag="alpha")
                V.tensor_scalar(out=alpha[:], in0=esc[:], scalar1=rcp[:, :1], scalar2=None, op0=Alu.mult)
                if taps:
                    nc.sync.dma_start(out=tap_d["alpha"][blk, :], in_=alpha[:])
                xjk_t = WK.tile([P, H], dt.float32, tag="xjkt")
                for l in range(L):
                    pstr = PSA.tile([P, P], dt.bfloat16, space="PSUM", tag="ps_a")
                    PE.transpose(out=pstr[:], in_=hjt[l][:], identity=ident_b[:])
                    if l == 0:
                        V.tensor_scalar(out=xjk_t[:], in0=pstr[:], scalar1=alpha[:, 0:1],
                                        scalar2=None, op0=Alu.mult)
                    else:
                        V.scalar_tensor_tensor(out=xjk_t[:], in0=pstr[:],
                                               scalar=alpha[:, l : l + 1], in1=xjk_t[:],
                                               op0=Alu.mult, op1=Alu.add)
                gt = WK.tile([P, P], dt.float32, tag="gonehot")
                V.tensor_scalar(out=gt[:], in0=iota_f[:], scalar1=col(batchloc, b),
                                scalar2=None, op0=Alu.is_equal)
                PE.matmul(out=psP[:], lhsT=gt[:], rhs=xjk_t[:],
                          start=(b == 0), stop=(b == NB - 1), skip_group_check=True)
                nc.sync.dma_start(out=g("xjk_dram")[blk, :], in_=xjk_t[:])

            if PH == 5:
                _dummy_out()
                return
            pool_sb = WK.tile([P, H], dt.float32, tag="poolsb")
            V.tensor_copy(out=pool_sb[:], in_=psP[:])
            if taps:
                nc.sync.dma_start(out=tap_d["pool"][:], in_=pool_sb[:])
            nc.gpsimd.indirect_dma_start(
                out=g("pool_in")[:, :],
                out_offset=bass.IndirectOffsetOnAxis(ap=gscat[:, :1], axis=0),
                in_=pool_sb[:], in_offset=None,
            )
            nc.gpsimd.collective_compute(
                "AllReduce", Alu.add, replica_groups=rg,
                ins=[g("pool_in")[:]], outs=[g("pool_out")[:]],
            )

            if PH == 6:
                _dummy_out()
                return
            # ---------- max pooling ---------------------------------------
            gm = [WK.tile([P, H], dt.float32, tag=f"gm{i}", name=f"gm{i}") for i in range(2)]
            for ch in range(CH):
                mg = MG.tile([P, 8, P], dt.float32, tag="maxg")
                nc.gpsimd.dma_gather(
                    out_ap=mg[:].bitcast(dt.bfloat16),
                    in_ap=g("xjk_dram")[:].bitcast(dt.bfloat16),
                    idxs_ap=maxidx[:, ch * 64 : (ch + 1) * 64],
                    num_idxs=8 * P, num_idxs_reg=8 * P, elem_size=2 * P,
                )
                mview = mg[:].rearrange("p s f -> p f s")
                if ch == 0:
                    V.reduce_max(out=gm[0][:], in_=mview, axis=mybir.AxisListType.X)
                else:
                    mx_ch = WK.tile([P, H], dt.float32, tag="mxch")
                    V.reduce_max(out=mx_ch[:], in_=mview, axis=mybir.AxisListType.X)
                    V.tensor_tensor(out=gm[ch % 2][:], in0=gm[(ch - 1) % 2][:],
                                    in1=mx_ch[:], op=Alu.max)
            gmax_f = gm[(CH - 1) % 2]
            if taps:
                nc.sync.dma_start(out=tap_d["gmax"][:], in_=gmax_f[:])
            nc.gpsimd.indirect_dma_start(
                out=g("max_in")[:, :],
                out_offset=bass.IndirectOffsetOnAxis(ap=gscat[:, :1], axis=0),
                in_=gmax_f[:], in_offset=None,
            )
            nc.gpsimd.collective_compute(
                "AllReduce", Alu.max, replica_groups=rg,
                ins=[g("max_in")[:]], outs=[g("max_out")[:]],
            )

            if PH == 7:
                _dummy_out()
                return
            # ---------- head ----------------------------------------------
            cmax = VW.tile([P, 4], dt.float32, tag="cmax")
            V.tensor_scalar(out=cmax[:], in0=cntc[:], scalar1=1.0, scalar2=None, op0=Alu.max)
            rc = VW.tile([P, 4], dt.float32, tag="rc")
            V.reciprocal(out=rc[:], in_=cmax[:])
            mask = VW.tile([P, 4], dt.float32, tag="mask")
            V.tensor_scalar(out=mask[:], in0=cntc[:], scalar1=0.0, scalar2=None, op0=Alu.is_gt)
            # pw softmax [1,3]
            pmax = VW.tile([1, 1], dt.float32, tag="pmax")
            V.reduce_max(out=pmax[:], in_=poolw[:], axis=mybir.AxisListType.X)
            npm = VW.tile([1, 1], dt.float32, tag="npm")
            V.tensor_scalar(out=npm[:], in0=pmax[:], scalar1=-1.0, scalar2=None, op0=Alu.mult)
            epw = VW.tile([1, 3], dt.float32, tag="epw")
            S.activation(out=epw[:], in_=poolw[:], func=Act.Exp, bias=npm[:, :1])
            spw = VW.tile([1, 1], dt.float32, tag="spw")
            V.reduce_sum(out=spw[:], in_=epw[:], axis=mybir.AxisListType.X)
            rpw = VW.tile([1, 1], dt.float32, tag="rpw")
            V.reciprocal(out=rpw[:], in_=spw[:])
            pwn = VW.tile([1, 3], dt.float32, tag="pwn")
            V.tensor_scalar(out=pwn[:], in0=epw[:], scalar1=rpw[:, :1], scalar2=None, op0=Alu.mult)
            pspw = PSB.tile([P, 3], dt.float32, space="PSUM", tag="ps_b")
            PE.matmul(out=pspw[:], lhsT=ones_r[:], rhs=pwn[:], start=True, stop=True)
            pw_bc = VW.tile([P, 3], dt.float32, tag="pwbc")
            V.tensor_copy(out=pw_bc[:], in_=pspw[:])

            pooledT = C.tile([P, G], dt.float32, tag="pooledT")
            for t in range(4):
                sl = slice(t * P, (t + 1) * P)
                sum_t = WK.tile([P, H], dt.float32, tag="sumt")
                nc.sync.dma_start(out=sum_t[:], in_=g("pool_out")[sl, :])
                mx_t = WK.tile([P, H], dt.float32, tag="mxt")
                nc.sync.dma_start(out=mx_t[:], in_=g("max_out")[sl, :])
                mean_t = WK.tile([P, H], dt.float32, tag="meant")
                V.tensor_scalar(out=mean_t[:], in0=sum_t[:], scalar1=col(rc, t),
                                scalar2=None, op0=Alu.mult)
                mxf_t = WK.tile([P, H], dt.float32, tag="mxft")
                V.tensor_scalar(out=mxf_t[:], in0=mx_t[:], scalar1=col(mask, t),
                                scalar2=None, op0=Alu.mult)
                acc1 = WK.tile([P, H], dt.float32, tag="acc1")
                V.tensor_scalar(out=acc1[:], in0=sum_t[:], scalar1=col(pw_bc, 0),
                                scalar2=None, op0=Alu.mult)
                acc2 = WK.tile([P, H], dt.float32, tag="acc2")
                V.scalar_tensor_tensor(out=acc2[:], in0=mean_t[:], scalar=col(pw_bc, 1),
                                       in1=acc1[:], op0=Alu.mult, op1=Alu.add)
                pooled_t = WK.tile([P, H], dt.float32, tag="pooledt")
                V.scalar_tensor_tensor(out=pooled_t[:], in0=mxf_t[:], scalar=col(pw_bc, 2),
                                       in1=acc2[:], op0=Alu.mult, op1=Alu.add)
                pstp = PSA.tile([P, P], dt.float32, space="PSUM", tag="ps_a")
                PE.transpose(out=pstp[:], in_=pooled_t[:], identity=ident_f[:])
                V.tensor_copy(out=pooledT[:, sl], in_=pstp[:])

            psO1 = PSZ.tile([P, G], dt.float32, space="PSUM", tag="ps_z")
            PE.matmul(out=psO1[:], lhsT=fcA[:], rhs=pooledT[:], start=True, stop=True)
            out1T = C.tile([P, G], dt.float32, tag="out1T")
            V.tensor_scalar(out=out1T[:], in0=psO1[:], scalar1=col(vecs, 18),
                            scalar2=None, op0=Alu.add)
            zT = C.tile([P, G], dt.float32, tag="zT")
            for t in range(4):
                sl = slice(t * P, (t + 1) * P)
                pstb = PSA.tile([P, P], dt.float32, space="PSUM", tag="ps_a")
                PE.transpose(out=pstb[:], in_=out1T[:, sl], identity=ident_f[:])
                o1t = WK.tile([P, H], dt.float32, tag="o1t")
                V.tensor_copy(out=o1t[:], in_=pstb[:])
                mt = vt()
                V.reduce_sum(out=mt[:], in_=o1t[:], axis=mybir.AxisListType.X)
                V.tensor_scalar(out=mt[:], in0=mt[:], scalar1=1.0 / H, scalar2=None, op0=Alu.mult)
                sq2 = WK.tile([P, H], dt.float32, tag="sq2")
                qt = vt()
                S.activation(out=sq2[:], in_=o1t[:], func=Act.Square, accum_out=qt[:])
                V.tensor_scalar(out=qt[:], in0=qt[:], scalar1=1.0 / H, scalar2=None, op0=Alu.mult)
                m2 = vt()
                V.tensor_tensor(out=m2[:], in0=mt[:], in1=mt[:], op=Alu.mult)
                V.tensor_tensor(out=qt[:], in0=qt[:], in1=m2[:], op=Alu.subtract)
                stdt = vt()
                S.activation(out=stdt[:], in_=qt[:], func=Act.Sqrt, bias=epsc[:, :1])
                rstdt = vt()
                V.reciprocal(out=rstdt[:], in_=stdt[:])
                z_t = WK.tile([P, H], dt.float32, tag="zt")
                V.tensor_scalar(out=z_t[:], in0=o1t[:], scalar1=mt[:, :1],
                                scalar2=rstdt[:, :1], op0=Alu.subtract, op1=Alu.mult)
                pstz = PSA.tile([P, P], dt.float32, space="PSUM", tag="ps_a")
                PE.transpose(out=pstz[:], in_=z_t[:], identity=ident_f[:])
                V.tensor_copy(out=zT[:, sl], in_=pstz[:])
            gT = C.tile([P, G], dt.float32, tag="gT")
            S.activation(out=gT[:], in_=zT[:], func=Act.Gelu,
                         bias=col(vecs, 20), scale=col(vecs, 19))
            V.tensor_tensor(out=gT[:], in0=gT[:], in1=pooledT[:], op=Alu.add)
            psO2 = PSZ.tile([LAT, G], dt.float32, space="PSUM", tag="ps_z")
            PE.matmul(out=psO2[:], lhsT=fcB[:], rhs=gT[:], start=True, stop=True)
            outT = C.tile([P, G], dt.float32, tag="outT")
            nc.vector.memset(outT[:], 0.0)
            V.tensor_scalar(out=outT[:LAT, :], in0=psO2[:], scalar1=fcBb[:, :1],
                            scalar2=None, op0=Alu.add)
            for t in range(4):
                sl = slice(t * P, (t + 1) * P)
                psf = PSA.tile([P, P], dt.float32, space="PSUM", tag="ps_a")
                PE.transpose(out=psf[:], in_=outT[:, sl], identity=ident_f[:])
                res_t = WK.tile([P, P], dt.float32, tag="rest")
                V.tensor_copy(out=res_t[:], in_=psf[:])
                nc.sync.dma_start(out=g("out_d")[sl, :], in_=res_t[:, :LAT])


# ---------------------------------------------------------------------------
# host entry point
# ---------------------------------------------------------------------------


def make_in_maps(consts, percore, x_raw, weights):
    vecs = np.stack(
        [
            weights["input_bn_g"], weights["input_bn_b"],
            *[weights["bn1_g"][l] for l in range(L)],
            *[weights["bn1_b"][l] for l in range(L)],
            *[weights["bn_g"][l] for l in range(L)],
            *[weights["bn_b"][l] for l in range(L)],
            weights["fcA_b"], weights["ln_g"], weights["ln_b"],
        ],
        axis=1,
    ).astype(np.float32)  # [128, 21]
    iota = np.tile(np.arange(P, dtype=np.float32), (P, 1))
    ident = np.eye(P, dtype=np.float32)
    shared = dict(
        x_raw=x_raw,
        w1=np.concatenate([weights["fc1_w"][l] for l in range(L)], axis=1).astype(np.float32),
        w2=np.concatenate([weights["fc2_w"][l] for l in range(L)], axis=1).astype(np.float32),
        vecs=vecs,
        attT=weights["att_weight"].T.astype(np.float32).copy(),
        eps_row=weights["eps"].reshape(1, L).astype(np.float32),
        poolw=weights["pool_weight"].reshape(1, 3).astype(np.float32),
        fcA=weights["fcA_w"].astype(np.float32),
        fcB=weights["fcB_w"].astype(np.float32),
        fcBb=weights["fcB_b"].reshape(LAT, 1).astype(np.float32),
        cnt_col=consts["cnt"].reshape(4, P).T.copy().astype(np.float32),
        ones_row=np.ones((1, P), np.float32),
        iota_f=iota,
        iota_b=iota.astype(bf16),
        ident_f=ident,
        ident_b=ident.astype(bf16),
    )
    in_maps = []
    for c in range(NCORES):
        m = dict(shared)
        m.update(percore[c])
        in_maps.append(m)
    return in_maps


_CACHE = {}


def kernel(**inputs):
    from concourse.bass_utils import run_bass_kernel_spmd

    x = np.asarray(inputs["x"], np.float32)
    consts, percore, x_raw = preprocess(x, inputs["edge_index"], inputs["batch"])
    key = (consts["TL"], consts["TH"], consts["W"])
    if key not in _CACHE:
        _CACHE[key] = build_bass(consts, taps=False)
    nc = _CACHE[key]
    weights = {k: np.asarray(v) for k, v in inputs.items()}
    in_maps = make_in_maps(consts, percore, x_raw, weights)
    res = run_bass_kernel_spmd(nc, in_maps, list(range(NCORES)))
    return np.asarray(res.results[0]["out"], np.float32)


if __name__ == "__main__":
    pass
